# revision 37
# baseline (speedup 1.0000x reference)
"""Multi-head attention TRN2 Bass kernel (8 NeuronCores), v2.

Problem: B=4, S=2048, D_MODEL=1024, H=16, d_k=d_v=64 (fp32 in/out).

Sharding: core c handles batch b=c//2 and head-half hh=c%2 (8 heads).
Each core computes partial_out = softmax(qh@khT/8) @ vh @ Wo[rows of its
heads]; the host sums the two partials per batch.

v2 structure (vs v1): the kernel is ACT(exp)-throughput-paced in steady
state (exp of 33.5M scores/core = 284us on ScalarE vs ~274us of PE
work), so everything else must hide under the exp stream:
  - blocks run PAIR-major ((sq0..3, pair0), (sq0..3, pair1), ...): the
    first exp only needs K-proj m0 + Q-proj m0-sb0, so the exp stream
    starts ~8us in (v1: 71.6us).
  - all other PE work (V projection, K/Q projections for later pairs,
    Wo for completed sq blocks) is chopped into 1-2-matmul "filler
    units" pumped between the skt iterations, filling the PE slack the
    ACT pace leaves without delaying the next scores. Fillers
    accumulate in a dedicated 2x1-bank PSUM pool (ps_fill) so their
    slot lifetimes never collide with the block accumulators.
  - PSUM budget: scps 2x[128,1024] + av 1x[128,1024] + fill
    2x[128,512] = 16KB/partition exactly. av is single-buffered; the
    WAR wait on the previous block's normalize is absorbed by pumping
    fillers at the block boundary.
  - host pre-packs every input into the exact SBUF staging layout so
    each DMA is one fully contiguous 1MB transfer (8KB/partition
    lines), spread over the sync/scalar/vector DMA queues in
    consumption order.
  - the vh ones-region memset runs on the idle GpSimd engine instead of
    blocking the Vector engine's projection casts.
Per-block math (scores transposed, row-group-packed K=64 pairs; AV with
the [ones64|vh] stationary giving the softmax denominator for free;
reciprocal+mul normalize into fp16 Wo stationaries) is unchanged.
"""

import numpy as np

import concourse.bass as bass  # noqa: F401
import concourse.mybir as mybir
import concourse.tile as tile
from concourse import bacc
from concourse.bass_utils import run_bass_kernel_spmd

S = 2048  # sequence length
D = 1024  # d_model
HPC = 8  # heads per core
DK = 64  # head dim
HD = HPC * DK  # 512: projected width per core
N_CORES = 8

SB = S // 512  # 4 s-blocks of 512
KT = D // 128  # 8 contraction tiles for projections
SKT = S // 128  # 16 key tiles
NP = HPC // 2  # 4 head pairs
F32 = mybir.dt.float32
F16 = mybir.dt.float16

_CACHE = {}


def _build(debug=False):
    nc = bacc.Bacc("TRN2", target_bir_lowering=False, debug=False, num_devices=N_CORES)
    # host-packed staging layouts: [sb, 128, KT, 512] so each sb is one
    # contiguous 1MB DMA; weights pre-packed to their SBUF layouts.
    kTp = nc.dram_tensor("kTp", [SB, 128, KT, 512], F16, kind="ExternalInput")
    qTp = nc.dram_tensor("qTp", [SB, 128, KT, 512], F16, kind="ExternalInput")
    vTp = nc.dram_tensor("vTp", [SB, 128, KT, 512], F16, kind="ExternalInput")
    # wq/wk are m-major so the m0 slice is one contiguous 256KB DMA
    wq = nc.dram_tensor("wq", [NP, 128, KT, 128], F16, kind="ExternalInput")
    wk = nc.dram_tensor("wk", [NP, 128, KT, 128], F16, kind="ExternalInput")
    wv = nc.dram_tensor("wv", [128, KT, HD], F16, kind="ExternalInput")
    wo = nc.dram_tensor("wo", [128, HD // 128, D], F16, kind="ExternalInput")
    out = nc.dram_tensor("out", [S, D], F16, kind="ExternalOutput")
    if debug:
        dbg_stk = nc.dram_tensor("dbg_stk", [SB, NP, 128, 512], F16, kind="ExternalOutput")
        dbg_vh = nc.dram_tensor("dbg_vh", [128, SKT, HPC, 128], F16, kind="ExternalOutput")
        dbg_kh0 = nc.dram_tensor("dbg_kh0", [128, S], F16, kind="ExternalOutput")
        dbg_qh0 = nc.dram_tensor("dbg_qh0", [128, S], F16, kind="ExternalOutput")
        dbg_kh3 = nc.dram_tensor("dbg_kh3", [128, S], F16, kind="ExternalOutput")
        dbg_qh3 = nc.dram_tensor("dbg_qh3", [128, S], F16, kind="ExternalOutput")

    with tile.TileContext(nc) as tc:
        with (
            tc.tile_pool(name="resident", bufs=1) as resident,
            tc.tile_pool(name="tstage", bufs=12) as tstage,
            tc.tile_pool(name="khp", bufs=2) as khpool,
            tc.tile_pool(name="qhp", bufs=2) as qhpool,
            tc.tile_pool(name="et", bufs=3) as etp,
            tc.tile_pool(name="rcp", bufs=1) as rcpp,
            tc.tile_pool(name="stk", bufs=16) as stkp,
            tc.tile_pool(name="outst", bufs=2) as outstp,
            tc.tile_pool(name="ps_sc", bufs=2, space="PSUM") as ps_sc,
            tc.tile_pool(name="ps_av", bufs=1, space="PSUM") as ps_av,
            tc.tile_pool(name="ps_fill", bufs=2, space="PSUM") as ps_fill,
        ):
            # --- resident tiles ---
            wv16 = resident.tile([128, KT, HD], F16)
            wk16 = resident.tile([128, NP, KT, 128], F16)
            wq16 = resident.tile([128, NP, KT, 128], F16)
            wo16 = resident.tile([128, HD // 128, D], F16)
            # AV stationary: [..., 0:64] = 1.0 (denominator), [..., 64:128] = vh
            vh = resident.tile([128, SKT, HPC, 128], F16)

            # --- staging tiles + DMA issue (consumption order, 3 queues) ---
            ksts = [
                tstage.tile([128, KT, 512], F16, tag="tstage", name=f"ksts{i}")
                for i in range(SB)
            ]
            qsts = [
                tstage.tile([128, KT, 512], F16, tag="tstage", name=f"qsts{i}")
                for i in range(SB)
            ]
            vsts = [
                tstage.tile([128, KT, 512], F16, tag="tstage", name=f"vsts{i}")
                for i in range(SB)
            ]

            # Per-queue DMA bandwidth is ~110GB/s, so the ~11.5MB the
            # pre-chain + early blocks need is spread across all three
            # queues in deadline order (K/Q/V first-wave in parallel).
            nc.sync.dma_start(out=wk16[:, 0], in_=wk.ap()[0])
            nc.sync.dma_start(out=ksts[0], in_=kTp.ap()[0])
            nc.sync.dma_start(out=vsts[1], in_=vTp.ap()[1])
            nc.sync.dma_start(out=ksts[2], in_=kTp.ap()[2])
            for m in range(1, NP):
                nc.sync.dma_start(out=wk16[:, m], in_=wk.ap()[m])
            nc.scalar.dma_start(out=wv16, in_=wv.ap())
            nc.scalar.dma_start(out=vsts[0], in_=vTp.ap()[0])
            nc.scalar.dma_start(out=vsts[3], in_=vTp.ap()[3])
            nc.scalar.dma_start(out=ksts[1], in_=kTp.ap()[1])
            nc.scalar.dma_start(out=qsts[1], in_=qTp.ap()[1])
            nc.scalar.dma_start(out=wo16, in_=wo.ap())
            nc.gpsimd.dma_start(out=wq16[:, 0], in_=wq.ap()[0])
            nc.gpsimd.dma_start(out=qsts[0], in_=qTp.ap()[0])
            nc.gpsimd.dma_start(out=vsts[2], in_=vTp.ap()[2])
            nc.gpsimd.dma_start(out=ksts[3], in_=kTp.ap()[3])
            nc.gpsimd.dma_start(out=qsts[2], in_=qTp.ap()[2])
            nc.gpsimd.dma_start(out=qsts[3], in_=qTp.ap()[3])
            for m in range(1, NP):
                nc.gpsimd.dma_start(out=wq16[:, m], in_=wq.ap()[m])
            # PE warm-up: ~28 matmuls on a junk tile, discarded into a
            # ps_fill slot. They run while the first DMAs land (PE would
            # idle) and keep the HAM clock gate at full rate for the
            # real pre-chain (else its first ~3.4us run at half clock).
            junk = resident.tile([128, 512], F16)
            nc.vector.memset(junk[:, :], 0.5)
            wps = ps_fill.tile([128, 512], F32, tag="fill", name="warm_ps")
            for _ in range(28):
                nc.tensor.matmul(
                    wps[:, :],
                    lhsT=junk[:, 0:128],
                    rhs=junk[:, :],
                    start=True,
                    stop=True,
                )
            # ones memsets on the Vector engine (DVE-write -> PE-read deps
            # are the well-trodden path). DVE is idle until the first
            # projection CAST, and these finish by ~8us.
            for c in range(SKT):
                nc.vector.memset(vh[:, c, :, 0:DK], 1.0)

            # --- projection chunk emitters (filler generators) ---
            def kq_proj_chunk(sts, w16, dst, m, sb):
                """One (m, sb) K/Q projection: 8 MMs + CAST into dst
                [128, 2048] at cols sb*512. Yields every 2 MMs."""
                ps = ps_fill.tile([128, 512], F32, tag="fill")
                for t in range(KT):
                    nc.tensor.matmul(
                        ps[:, :],
                        lhsT=w16[:, m, t, :],
                        rhs=sts[sb][:, t, :],
                        start=(t == 0),
                        stop=(t == KT - 1),
                    )
                    if t % 2 == 1 and t < KT - 1:
                        yield
                nc.vector.tensor_copy(dst[:, sb * 512 : (sb + 1) * 512], ps[:, :])
                yield

            def v_chunk(c):
                """One vh chunk (sb=c//4, cc=c%4): 8 MMs + CAST into
                vh[:, c, :, 64:128]. Yields every 2 MMs."""
                sb, cc = c // 4, c % 4
                ps = ps_fill.tile([128, 512], F32, tag="fill")
                for t in range(KT):
                    nc.tensor.matmul(
                        ps[:, :],
                        lhsT=vsts[sb][:, t, cc * 128 : (cc + 1) * 128],
                        rhs=wv16[:, t, :],
                        start=(t == 0),
                        stop=(t == KT - 1),
                    )
                    if t % 2 == 1 and t < KT - 1:
                        yield
                nc.vector.tensor_copy(
                    vh[:, c, :, DK:128],
                    ps[:, :].rearrange("p (h d) -> p h d", h=HPC),
                )
                yield

            def wo_chunk(sq, chunk, stks):
                """One Wo half-chunk pipeline: 128 out rows, both nh
                halves; 8 MMs total, CAST+DMA per nh. Yields every MM."""
                outst = outstp.tile([128, 2, 512], F16)
                mrange = slice(chunk * 128, (chunk + 1) * 128)
                for nh in range(2):
                    wops = ps_fill.tile([128, 512], F32, tag="fill")
                    for pair in range(NP):
                        nc.tensor.matmul(
                            wops[:, :],
                            lhsT=stks[pair][:, mrange],
                            rhs=wo16[:, pair, nh * 512 : (nh + 1) * 512],
                            start=(pair == 0),
                            stop=(pair == NP - 1),
                        )
                        if pair % 2 == 1 and not (nh == 1 and pair == NP - 1):
                            yield
                    nc.vector.tensor_copy(outst[:, nh, :], wops[:, :])
                row0 = sq * 512 + chunk * 128
                nc.sync.dma_start(
                    out=out.ap()[row0 : row0 + 128, :],
                    in_=outst.rearrange("p a b -> p (a b)"),
                )
                yield

            # --- filler machinery ---
            fillers = []  # list of (gate_block_idx, generator)

            def pump(cur_block, budget=1):
                done, i = 0, 0
                while done < budget and i < len(fillers):
                    gate, gen = fillers[i]
                    if gate > cur_block:
                        i += 1
                        continue
                    try:
                        next(gen)
                        done += 1
                    except StopIteration:
                        fillers.pop(i)
                return done

            def flush():
                while fillers:
                    _, gen = fillers.pop(0)
                    for _ in gen:
                        pass

            # --- attention block (inline finish) ---
            def av_mms(et, skt, av, pair):
                for x in range(2):
                    nc.tensor.matmul(
                        av[:, x * 512 : (x + 1) * 512],
                        lhsT=vh[:, skt, 2 * pair + x, :],
                        rhs=et[:, x, :],
                        start=(skt == 0),
                        stop=(skt == SKT - 1),
                    )

            def attention_block(block_idx, sq, pair, kh, qh, stks_by_sq):
                cols = slice(sq * 512, (sq + 1) * 512)
                budget = 1
                av = ps_av.tile([128, 1024], F32, tag="av")
                prev = None
                for skt in range(SKT):
                    scps = ps_sc.tile([128, 1024], F32, tag="sc")
                    kcols = slice(skt * 128, (skt + 1) * 128)
                    nc.tensor.matmul(
                        scps[:, 0:512],
                        lhsT=kh[0:64, kcols],
                        rhs=qh[0:64, cols],
                        start=True,
                        stop=True,
                    )
                    nc.tensor.matmul(
                        scps[:, 512:1024],
                        lhsT=kh[64:128, kcols],
                        rhs=qh[64:128, cols],
                        start=True,
                        stop=True,
                    )
                    if prev is not None:
                        av_mms(*prev)
                    pump(block_idx, budget)
                    et = etp.tile([128, 2, 512], F16)
                    nc.scalar.activation(
                        et.rearrange("p a b -> p (a b)"),
                        scps[:, :],
                        mybir.ActivationFunctionType.Exp,
                        scale=1.0 / np.sqrt(DK).item(),
                    )
                    prev = (et, skt, av, pair)
                # fillers cover the wait for the block's last exp
                pump(block_idx, 2)
                av_mms(*prev)
                # normalize: 1/r (rows 0:64) -> scale out rows into fp16
                # Wo stationary
                rcp = rcpp.tile([128, 1024], F32, tag="rcp")
                nc.vector.reciprocal_approx_fast(out=rcp[0:64, :], in_=av[0:64, :])
                stk = stkp.tile([128, 512], F16, tag="stk")
                nc.vector.tensor_mul(stk[0:64, :], av[64:128, 0:512], rcp[0:64, 0:512])
                nc.vector.tensor_mul(
                    stk[64:128, :], av[64:128, 512:1024], rcp[0:64, 512:1024]
                )
                stks_by_sq[sq][pair] = stk
                # boundary pump: runs during the normalize chain, so the
                # single-buffered av slot is free again by the time the
                # next block's first AV matmul needs it. Bigger in the
                # pair-3 window so the Wo backlog drains before the tail.
                pump(block_idx, 6 if block_idx >= 12 else 3)

            # --- pre-chain: K m0 sb0 + Q m0 sb0, then the FULL V
            # projection. V must complete before pair0's AV matmuls
            # consume vh at the exp pace; projecting it up front costs
            # ~27us that overlaps the kT/qT DMA stream, and removes the
            # in-block starvation stalls (and the buffer-recycling
            # pressure they caused). ---
            khts = {}
            qhts = {}
            khts[0] = khpool.tile([128, S], F16, tag="khp", name="kht0")
            qhts[0] = qhpool.tile([128, S], F16, tag="qhp", name="qht0")
            for _ in kq_proj_chunk(ksts, wk16, khts[0], 0, 0):
                pass
            for _ in kq_proj_chunk(qsts, wq16, qhts[0], 0, 0):
                pass
            for c in range(SKT):
                for _ in v_chunk(c):
                    pass

            # --- filler list (priority order == deadline order) ---
            # K m0 sb1-3 (block 0 consumes khT[0] cols at 1 sb per 4 skt
            # iters), Q m0 sb1-3 (blocks 1-3), then per pair m: K all sb
            # + Q sb0 (due at block 4m), then Q sb1-3 (blocks 4m+1..3).
            for sb in range(1, SB):
                fillers.append((0, kq_proj_chunk(ksts, wk16, khts[0], 0, sb)))
            for sb in range(1, SB):
                fillers.append((0, kq_proj_chunk(qsts, wq16, qhts[0], 0, sb)))

            def m_proj_stream(m):
                """K (all sb) + Q (sb0) projections for pair m. The kh/qh
                pool tiles are allocated lazily at first pump (during
                window m-1) so the slot they recycle (pair m-2's) has all
                its reads emitted."""
                kh = khpool.tile([128, S], F16, tag="khp", name=f"kht{m}")
                qh = qhpool.tile([128, S], F16, tag="qhp", name=f"qht{m}")
                khts[m] = kh
                qhts[m] = qh
                for sb in range(SB):
                    yield from kq_proj_chunk(ksts, wk16, kh, m, sb)
                yield from kq_proj_chunk(qsts, wq16, qh, m, 0)

            def q_rest_stream(m):
                for sb in range(1, SB):
                    yield from kq_proj_chunk(qsts, wq16, qhts[m], m, sb)

            m_streams = {}
            for m in range(1, NP):
                g = m_proj_stream(m)
                m_streams[m] = g
                fillers.append((0, g))
                fillers.append((0, q_rest_stream(m)))

            def ensure_pair_ready(m):
                """Force-finish pair m's projection stream if the filler
                pump hasn't consumed it yet (safety net)."""
                if m == 0:
                    return
                g = m_streams[m]
                for i, (_, gen) in enumerate(fillers):
                    if gen is g:
                        for _ in g:
                            pass
                        fillers.pop(i)
                        break

            stks_by_sq = [[None] * NP for _ in range(SB)]

            # --- the block chain, pair-major ---
            block_idx = 0
            for pair in range(NP):
                ensure_pair_ready(pair)
                if debug and pair == 1:
                    # snapshot pair0's projections before slot recycling
                    nc.scalar.dma_start(out=dbg_kh0.ap(), in_=khts[0][:, :])
                    nc.scalar.dma_start(out=dbg_qh0.ap(), in_=qhts[0][:, :])
                for sq in range(SB):
                    if pair == NP - 1 and sq >= 1:
                        # previous sq's last pair is done: queue its Wo
                        # into the filler stream
                        for chunk in range(4):
                            fillers.append(
                                (block_idx, wo_chunk(sq - 1, chunk, stks_by_sq[sq - 1]))
                            )
                    attention_block(
                        block_idx, sq, pair, khts[pair], qhts[pair], stks_by_sq
                    )
                    block_idx += 1

            flush()
            # tail: Wo for the last sq block
            for chunk in range(4):
                for _ in wo_chunk(SB - 1, chunk, stks_by_sq[SB - 1]):
                    pass
            if debug:
                for sq in range(SB):
                    for pair in range(NP):
                        nc.sync.dma_start(
                            out=dbg_stk.ap()[sq, pair],
                            in_=stks_by_sq[sq][pair][:, :],
                        )
                nc.sync.dma_start(out=dbg_vh.ap(), in_=vh[:, :, :, :])
                nc.sync.dma_start(out=dbg_kh3.ap(), in_=khts[3][:, :])
                nc.sync.dma_start(out=dbg_qh3.ap(), in_=qhts[3][:, :])

    nc.compile()
    return nc


def _get_nc():
    if "nc" not in _CACHE:
        _CACHE["nc"] = _build()
    return _CACHE["nc"]


def _pack_stage(aT):
    """[D, S] fp16 -> [SB, 128, KT, 512] contiguous staging layout."""
    # row t*128+p, col sb*512+s  ->  [sb, p, t, s]
    a = aT.reshape(KT, 128, SB, 512)  # [t, p, sb, s]
    return np.ascontiguousarray(a.transpose(2, 1, 0, 3))


def _pack_w(w):
    """[D, HD] fp16 -> [128, KT, HD] (row t*128+p -> [p, t, :])."""
    return np.ascontiguousarray(w.reshape(KT, 128, HD).transpose(1, 0, 2))


def _pack_w_m(w):
    """[D, HD] fp16 -> [NP, 128, KT, 128] (m-major pair chunks)."""
    return np.ascontiguousarray(w.reshape(KT, 128, NP, 128).transpose(2, 1, 0, 3))


def _pack_wo(w):
    """[HD, D] fp16 -> [128, 4, D]."""
    return np.ascontiguousarray(w.reshape(HD // 128, 128, D).transpose(1, 0, 2))


def build_in_maps(q, k, v, Wq, Wk, Wv, Wo):
    """Host prep: shard, cast fp16, pre-transpose + pack to DMA layouts."""
    q = np.asarray(q, dtype=np.float32)
    k = np.asarray(k, dtype=np.float32)
    v = np.asarray(v, dtype=np.float32)
    wq16 = np.asarray(Wq, dtype=np.float32).astype(np.float16)
    wk16 = np.asarray(Wk, dtype=np.float32).astype(np.float16)
    wv16 = np.asarray(Wv, dtype=np.float32).astype(np.float16)
    wo16 = np.asarray(Wo, dtype=np.float32).astype(np.float16)
    qTp = [_pack_stage(q[b].T.astype(np.float16)) for b in range(4)]
    kTp = [_pack_stage(k[b].T.astype(np.float16)) for b in range(4)]
    vTp = [_pack_stage(v[b].T.astype(np.float16)) for b in range(4)]
    in_maps = []
    for c in range(N_CORES):
        b, hh = c // 2, c % 2
        sl = slice(hh * HD, (hh + 1) * HD)
        in_maps.append(
            {
                "qTp": qTp[b],
                "kTp": kTp[b],
                "vTp": vTp[b],
                "wq": _pack_w_m(np.ascontiguousarray(wq16[:, sl])),
                "wk": _pack_w_m(np.ascontiguousarray(wk16[:, sl])),
                "wv": _pack_w(np.ascontiguousarray(wv16[:, sl])),
                "wo": _pack_wo(np.ascontiguousarray(wo16[sl, :])),
            }
        )
    return in_maps


def kernel(q, k, v, Wq, Wk, Wv, Wo):
    nc = _get_nc()
    in_maps = build_in_maps(q, k, v, Wq, Wk, Wv, Wo)
    res = run_bass_kernel_spmd(nc, in_maps, core_ids=list(range(N_CORES)))
    outs = [res.results[c]["out"].astype(np.float32) for c in range(N_CORES)]
    return np.stack([outs[2 * b] + outs[2 * b + 1] for b in range(4)], axis=0)


# revision 40
# speedup vs baseline: 1.0198x; 1.0198x over previous
"""Multi-head attention TRN2 Bass kernel (8 NeuronCores), v2.

Problem: B=4, S=2048, D_MODEL=1024, H=16, d_k=d_v=64 (fp32 in/out).

Sharding: core c handles batch b=c//2 and head-half hh=c%2 (8 heads).
Each core computes partial_out = softmax(qh@khT/8) @ vh @ Wo[rows of its
heads]; the host sums the two partials per batch.

v2 structure (vs v1): the kernel is ACT(exp)-throughput-paced in steady
state (exp of 33.5M scores/core = 284us on ScalarE vs ~274us of PE
work), so everything else must hide under the exp stream:
  - blocks run PAIR-major ((sq0..3, pair0), (sq0..3, pair1), ...): the
    first exp only needs K-proj m0 + Q-proj m0-sb0, so the exp stream
    starts ~8us in (v1: 71.6us).
  - all other PE work (V projection, K/Q projections for later pairs,
    Wo for completed sq blocks) is chopped into 1-2-matmul "filler
    units" pumped between the skt iterations, filling the PE slack the
    ACT pace leaves without delaying the next scores. Fillers
    accumulate in a dedicated 2x1-bank PSUM pool (ps_fill) so their
    slot lifetimes never collide with the block accumulators.
  - PSUM budget: scps 2x[128,1024] + av 1x[128,1024] + fill
    2x[128,512] = 16KB/partition exactly. av is single-buffered; the
    WAR wait on the previous block's normalize is absorbed by pumping
    fillers at the block boundary.
  - host pre-packs every input into the exact SBUF staging layout so
    each DMA is one fully contiguous 1MB transfer (8KB/partition
    lines), spread over the sync/scalar/vector DMA queues in
    consumption order.
  - the vh ones-region memset runs on the idle GpSimd engine instead of
    blocking the Vector engine's projection casts.
Per-block math (scores transposed, row-group-packed K=64 pairs; AV with
the [ones64|vh] stationary giving the softmax denominator for free;
reciprocal+mul normalize into fp16 Wo stationaries) is unchanged.
"""

import numpy as np

import concourse.bass as bass  # noqa: F401
import concourse.mybir as mybir
import concourse.tile as tile
from concourse import bacc
from concourse.bass_utils import run_bass_kernel_spmd

S = 2048  # sequence length
D = 1024  # d_model
HPC = 8  # heads per core
DK = 64  # head dim
HD = HPC * DK  # 512: projected width per core
N_CORES = 8

SB = S // 512  # 4 s-blocks of 512
KT = D // 128  # 8 contraction tiles for projections
SKT = S // 128  # 16 key tiles
NP = HPC // 2  # 4 head pairs
F32 = mybir.dt.float32
F16 = mybir.dt.float16

_CACHE = {}


def _build(debug=False):
    nc = bacc.Bacc("TRN2", target_bir_lowering=False, debug=False, num_devices=N_CORES)
    # host-packed staging layouts: [sb, 128, KT, 512] so each sb is one
    # contiguous 1MB DMA; weights pre-packed to their SBUF layouts.
    kTp = nc.dram_tensor("kTp", [SB, 128, KT, 512], F16, kind="ExternalInput")
    qTp = nc.dram_tensor("qTp", [SB, 128, KT, 512], F16, kind="ExternalInput")
    vTp = nc.dram_tensor("vTp", [SB, 128, KT, 512], F16, kind="ExternalInput")
    # wq/wk are m-major so the m0 slice is one contiguous 256KB DMA
    wq = nc.dram_tensor("wq", [NP, 128, KT, 128], F16, kind="ExternalInput")
    wk = nc.dram_tensor("wk", [NP, 128, KT, 128], F16, kind="ExternalInput")
    wv = nc.dram_tensor("wv", [128, KT, HD], F16, kind="ExternalInput")
    wo = nc.dram_tensor("wo", [128, HD // 128, D], F16, kind="ExternalInput")
    out = nc.dram_tensor("out", [S, D], F16, kind="ExternalOutput")
    if debug:
        dbg_stk = nc.dram_tensor("dbg_stk", [SB, NP, 128, 512], F16, kind="ExternalOutput")
        dbg_vh = nc.dram_tensor("dbg_vh", [128, SKT, HPC, 128], F16, kind="ExternalOutput")
        dbg_kh0 = nc.dram_tensor("dbg_kh0", [128, S], F16, kind="ExternalOutput")
        dbg_qh0 = nc.dram_tensor("dbg_qh0", [128, S], F16, kind="ExternalOutput")
        dbg_kh3 = nc.dram_tensor("dbg_kh3", [128, S], F16, kind="ExternalOutput")
        dbg_qh3 = nc.dram_tensor("dbg_qh3", [128, S], F16, kind="ExternalOutput")

    with tile.TileContext(nc) as tc:
        with (
            tc.tile_pool(name="resident", bufs=1) as resident,
            tc.tile_pool(name="tstage", bufs=12) as tstage,
            tc.tile_pool(name="khp", bufs=2) as khpool,
            tc.tile_pool(name="qhp", bufs=2) as qhpool,
            tc.tile_pool(name="et", bufs=3) as etp,
            tc.tile_pool(name="rcp", bufs=1) as rcpp,
            tc.tile_pool(name="stk", bufs=16) as stkp,
            tc.tile_pool(name="outst", bufs=2) as outstp,
            tc.tile_pool(name="ps_sc", bufs=2, space="PSUM") as ps_sc,
            tc.tile_pool(name="ps_av", bufs=1, space="PSUM") as ps_av,
            tc.tile_pool(name="ps_fill", bufs=2, space="PSUM") as ps_fill,
        ):
            # --- resident tiles ---
            wv16 = resident.tile([128, KT, HD], F16)
            wk16 = resident.tile([128, NP, KT, 128], F16)
            wq16 = resident.tile([128, NP, KT, 128], F16)
            wo16 = resident.tile([128, HD // 128, D], F16)
            # AV stationary: [..., 0:64] = 1.0 (denominator), [..., 64:128] = vh
            vh = resident.tile([128, SKT, HPC, 128], F16)

            # --- staging tiles + DMA issue (consumption order, 3 queues) ---
            ksts = [
                tstage.tile([128, KT, 512], F16, tag="tstage", name=f"ksts{i}")
                for i in range(SB)
            ]
            qsts = [
                tstage.tile([128, KT, 512], F16, tag="tstage", name=f"qsts{i}")
                for i in range(SB)
            ]
            vsts = [
                tstage.tile([128, KT, 512], F16, tag="tstage", name=f"vsts{i}")
                for i in range(SB)
            ]

            # Per-queue DMA bandwidth is ~110GB/s, so the ~11.5MB the
            # pre-chain + early blocks need is spread across all three
            # queues in deadline order (K/Q/V first-wave in parallel).
            nc.sync.dma_start(out=wk16[:, 0], in_=wk.ap()[0])
            nc.sync.dma_start(out=ksts[0], in_=kTp.ap()[0])
            nc.sync.dma_start(out=vsts[1], in_=vTp.ap()[1])
            nc.sync.dma_start(out=ksts[2], in_=kTp.ap()[2])
            for m in range(1, NP):
                nc.sync.dma_start(out=wk16[:, m], in_=wk.ap()[m])
            nc.scalar.dma_start(out=wv16, in_=wv.ap())
            nc.scalar.dma_start(out=vsts[0], in_=vTp.ap()[0])
            nc.scalar.dma_start(out=vsts[3], in_=vTp.ap()[3])
            nc.scalar.dma_start(out=ksts[1], in_=kTp.ap()[1])
            nc.scalar.dma_start(out=qsts[1], in_=qTp.ap()[1])
            nc.scalar.dma_start(out=wo16, in_=wo.ap())
            nc.gpsimd.dma_start(out=wq16[:, 0], in_=wq.ap()[0])
            nc.gpsimd.dma_start(out=vsts[2], in_=vTp.ap()[2])
            nc.gpsimd.dma_start(out=qsts[0], in_=qTp.ap()[0])
            nc.gpsimd.dma_start(out=ksts[3], in_=kTp.ap()[3])
            nc.gpsimd.dma_start(out=qsts[2], in_=qTp.ap()[2])
            nc.gpsimd.dma_start(out=qsts[3], in_=qTp.ap()[3])
            for m in range(1, NP):
                nc.gpsimd.dma_start(out=wq16[:, m], in_=wq.ap()[m])
            # PE warm-up: ~28 matmuls on a junk tile, discarded into a
            # ps_fill slot. They run while the first DMAs land (PE would
            # idle) and keep the HAM clock gate at full rate for the
            # real pre-chain (else its first ~3.4us run at half clock).
            junk = resident.tile([128, 512], F16)
            nc.vector.memset(junk[:, :], 0.5)
            wps = ps_fill.tile([128, 512], F32, tag="fill", name="warm_ps")
            for _ in range(24):
                nc.tensor.matmul(
                    wps[:, :],
                    lhsT=junk[:, 0:128],
                    rhs=junk[:, :],
                    start=True,
                    stop=True,
                )
            # ones memsets on the Vector engine (DVE-write -> PE-read deps
            # are the well-trodden path). DVE is idle until the first
            # projection CAST, and these finish by ~8us.
            for c in range(SKT):
                nc.vector.memset(vh[:, c, :, 0:DK], 1.0)

            # --- projection chunk emitters (filler generators) ---
            def kq_proj_chunk(sts, w16, dst, m, sb):
                """One (m, sb) K/Q projection: 8 MMs + CAST into dst
                [128, 2048] at cols sb*512. Yields every 2 MMs."""
                ps = ps_fill.tile([128, 512], F32, tag="fill")
                for t in range(KT):
                    nc.tensor.matmul(
                        ps[:, :],
                        lhsT=w16[:, m, t, :],
                        rhs=sts[sb][:, t, :],
                        start=(t == 0),
                        stop=(t == KT - 1),
                    )
                    if t % 2 == 1 and t < KT - 1:
                        yield
                nc.vector.tensor_copy(dst[:, sb * 512 : (sb + 1) * 512], ps[:, :])
                yield

            def v_chunk(c):
                """One vh chunk (sb=c//4, cc=c%4): 8 MMs + CAST into
                vh[:, c, :, 64:128]. Yields every 2 MMs."""
                sb, cc = c // 4, c % 4
                ps = ps_fill.tile([128, 512], F32, tag="fill")
                for t in range(KT):
                    nc.tensor.matmul(
                        ps[:, :],
                        lhsT=vsts[sb][:, t, cc * 128 : (cc + 1) * 128],
                        rhs=wv16[:, t, :],
                        start=(t == 0),
                        stop=(t == KT - 1),
                    )
                    if t % 2 == 1 and t < KT - 1:
                        yield
                nc.vector.tensor_copy(
                    vh[:, c, :, DK:128],
                    ps[:, :].rearrange("p (h d) -> p h d", h=HPC),
                )
                yield

            def wo_chunk(sq, chunk, stks):
                """One Wo half-chunk pipeline: 128 out rows, both nh
                halves; 8 MMs total, CAST+DMA per nh. Yields every MM."""
                outst = outstp.tile([128, 2, 512], F16)
                mrange = slice(chunk * 128, (chunk + 1) * 128)
                for nh in range(2):
                    wops = ps_fill.tile([128, 512], F32, tag="fill")
                    for pair in range(NP):
                        nc.tensor.matmul(
                            wops[:, :],
                            lhsT=stks[pair][:, mrange],
                            rhs=wo16[:, pair, nh * 512 : (nh + 1) * 512],
                            start=(pair == 0),
                            stop=(pair == NP - 1),
                        )
                        if pair % 2 == 1 and not (nh == 1 and pair == NP - 1):
                            yield
                    nc.vector.tensor_copy(outst[:, nh, :], wops[:, :])
                row0 = sq * 512 + chunk * 128
                nc.sync.dma_start(
                    out=out.ap()[row0 : row0 + 128, :],
                    in_=outst.rearrange("p a b -> p (a b)"),
                )
                yield

            # --- filler machinery ---
            fillers = []  # list of (gate_block_idx, generator)

            def pump(cur_block, budget=1):
                done, i = 0, 0
                while done < budget and i < len(fillers):
                    gate, gen = fillers[i]
                    if gate > cur_block:
                        i += 1
                        continue
                    try:
                        next(gen)
                        done += 1
                    except StopIteration:
                        fillers.pop(i)
                return done

            def flush():
                while fillers:
                    _, gen = fillers.pop(0)
                    for _ in gen:
                        pass

            # --- attention block (inline finish) ---
            def av_mms(et, skt, av, pair):
                for x in range(2):
                    nc.tensor.matmul(
                        av[:, x * 512 : (x + 1) * 512],
                        lhsT=vh[:, skt, 2 * pair + x, :],
                        rhs=et[:, x, :],
                        start=(skt == 0),
                        stop=(skt == SKT - 1),
                    )

            def attention_block(block_idx, sq, pair, kh, qh, stks_by_sq):
                cols = slice(sq * 512, (sq + 1) * 512)
                budget = 1
                av = ps_av.tile([128, 1024], F32, tag="av")
                prev = None
                for skt in range(SKT):
                    scps = ps_sc.tile([128, 1024], F32, tag="sc")
                    kcols = slice(skt * 128, (skt + 1) * 128)
                    nc.tensor.matmul(
                        scps[:, 0:512],
                        lhsT=kh[0:64, kcols],
                        rhs=qh[0:64, cols],
                        start=True,
                        stop=True,
                    )
                    nc.tensor.matmul(
                        scps[:, 512:1024],
                        lhsT=kh[64:128, kcols],
                        rhs=qh[64:128, cols],
                        start=True,
                        stop=True,
                    )
                    if prev is not None:
                        av_mms(*prev)
                    pump(block_idx, budget)
                    et = etp.tile([128, 2, 512], F16)
                    nc.scalar.activation(
                        et.rearrange("p a b -> p (a b)"),
                        scps[:, :],
                        mybir.ActivationFunctionType.Exp,
                        scale=1.0 / np.sqrt(DK).item(),
                    )
                    prev = (et, skt, av, pair)
                # fillers cover the wait for the block's last exp
                pump(block_idx, 2)
                av_mms(*prev)
                # normalize: 1/r (rows 0:64) -> scale out rows into fp16
                # Wo stationary
                rcp = rcpp.tile([128, 1024], F32, tag="rcp")
                nc.vector.reciprocal_approx_fast(out=rcp[0:64, :], in_=av[0:64, :])
                stk = stkp.tile([128, 512], F16, tag="stk")
                nc.vector.tensor_mul(stk[0:64, :], av[64:128, 0:512], rcp[0:64, 0:512])
                nc.vector.tensor_mul(
                    stk[64:128, :], av[64:128, 512:1024], rcp[0:64, 512:1024]
                )
                stks_by_sq[sq][pair] = stk
                # boundary pump: runs during the normalize chain, so the
                # single-buffered av slot is free again by the time the
                # next block's first AV matmul needs it. Bigger in the
                # pair-3 window so the Wo backlog drains before the tail.
                pump(block_idx, 6 if block_idx >= 12 else 3)

            # --- pre-chain: K m0 sb0 + Q m0 sb0, then the FULL V
            # projection. V must complete before pair0's AV matmuls
            # consume vh at the exp pace; projecting it up front costs
            # ~27us that overlaps the kT/qT DMA stream, and removes the
            # in-block starvation stalls (and the buffer-recycling
            # pressure they caused). ---
            khts = {}
            qhts = {}
            khts[0] = khpool.tile([128, S], F16, tag="khp", name="kht0")
            qhts[0] = qhpool.tile([128, S], F16, tag="qhp", name="qht0")
            # K first (kT0 is the first-wave DMA), V while its tiles
            # stream in, Q last (its deadline is right before the first
            # scores; its DMA rides the slow gpsimd queue)
            for _ in kq_proj_chunk(ksts, wk16, khts[0], 0, 0):
                pass
            for c in range(SKT):
                for _ in v_chunk(c):
                    pass
            for _ in kq_proj_chunk(qsts, wq16, qhts[0], 0, 0):
                pass

            # --- filler list (priority order == deadline order) ---
            # K m0 sb1-3 (block 0 consumes khT[0] cols at 1 sb per 4 skt
            # iters), Q m0 sb1-3 (blocks 1-3), then per pair m: K all sb
            # + Q sb0 (due at block 4m), then Q sb1-3 (blocks 4m+1..3).
            for sb in range(1, SB):
                fillers.append((0, kq_proj_chunk(ksts, wk16, khts[0], 0, sb)))
            for sb in range(1, SB):
                fillers.append((0, kq_proj_chunk(qsts, wq16, qhts[0], 0, sb)))

            def m_proj_stream(m):
                """K (all sb) + Q (sb0) projections for pair m. The kh/qh
                pool tiles are allocated lazily at first pump (during
                window m-1) so the slot they recycle (pair m-2's) has all
                its reads emitted."""
                kh = khpool.tile([128, S], F16, tag="khp", name=f"kht{m}")
                qh = qhpool.tile([128, S], F16, tag="qhp", name=f"qht{m}")
                khts[m] = kh
                qhts[m] = qh
                for sb in range(SB):
                    yield from kq_proj_chunk(ksts, wk16, kh, m, sb)
                yield from kq_proj_chunk(qsts, wq16, qh, m, 0)

            def q_rest_stream(m):
                for sb in range(1, SB):
                    yield from kq_proj_chunk(qsts, wq16, qhts[m], m, sb)

            m_streams = {}
            for m in range(1, NP):
                g = m_proj_stream(m)
                m_streams[m] = g
                fillers.append((0, g))
                fillers.append((0, q_rest_stream(m)))

            def ensure_pair_ready(m):
                """Force-finish pair m's projection stream if the filler
                pump hasn't consumed it yet (safety net)."""
                if m == 0:
                    return
                g = m_streams[m]
                for i, (_, gen) in enumerate(fillers):
                    if gen is g:
                        for _ in g:
                            pass
                        fillers.pop(i)
                        break

            stks_by_sq = [[None] * NP for _ in range(SB)]

            # --- the block chain, pair-major ---
            block_idx = 0
            for pair in range(NP):
                ensure_pair_ready(pair)
                if debug and pair == 1:
                    # snapshot pair0's projections before slot recycling
                    nc.scalar.dma_start(out=dbg_kh0.ap(), in_=khts[0][:, :])
                    nc.scalar.dma_start(out=dbg_qh0.ap(), in_=qhts[0][:, :])
                for sq in range(SB):
                    if pair == NP - 1 and sq >= 1:
                        # previous sq's last pair is done: queue its Wo
                        # into the filler stream
                        for chunk in range(4):
                            fillers.append(
                                (block_idx, wo_chunk(sq - 1, chunk, stks_by_sq[sq - 1]))
                            )
                    attention_block(
                        block_idx, sq, pair, khts[pair], qhts[pair], stks_by_sq
                    )
                    block_idx += 1

            flush()
            # tail: Wo for the last sq block
            for chunk in range(4):
                for _ in wo_chunk(SB - 1, chunk, stks_by_sq[SB - 1]):
                    pass
            if debug:
                for sq in range(SB):
                    for pair in range(NP):
                        nc.sync.dma_start(
                            out=dbg_stk.ap()[sq, pair],
                            in_=stks_by_sq[sq][pair][:, :],
                        )
                nc.sync.dma_start(out=dbg_vh.ap(), in_=vh[:, :, :, :])
                nc.sync.dma_start(out=dbg_kh3.ap(), in_=khts[3][:, :])
                nc.sync.dma_start(out=dbg_qh3.ap(), in_=qhts[3][:, :])

    nc.compile()
    return nc


def _get_nc():
    if "nc" not in _CACHE:
        _CACHE["nc"] = _build()
    return _CACHE["nc"]


def _pack_stage(aT):
    """[D, S] fp16 -> [SB, 128, KT, 512] contiguous staging layout."""
    # row t*128+p, col sb*512+s  ->  [sb, p, t, s]
    a = aT.reshape(KT, 128, SB, 512)  # [t, p, sb, s]
    return np.ascontiguousarray(a.transpose(2, 1, 0, 3))


def _pack_w(w):
    """[D, HD] fp16 -> [128, KT, HD] (row t*128+p -> [p, t, :])."""
    return np.ascontiguousarray(w.reshape(KT, 128, HD).transpose(1, 0, 2))


def _pack_w_m(w):
    """[D, HD] fp16 -> [NP, 128, KT, 128] (m-major pair chunks)."""
    return np.ascontiguousarray(w.reshape(KT, 128, NP, 128).transpose(2, 1, 0, 3))


def _pack_wo(w):
    """[HD, D] fp16 -> [128, 4, D]."""
    return np.ascontiguousarray(w.reshape(HD // 128, 128, D).transpose(1, 0, 2))


def build_in_maps(q, k, v, Wq, Wk, Wv, Wo):
    """Host prep: shard, cast fp16, pre-transpose + pack to DMA layouts."""
    q = np.asarray(q, dtype=np.float32)
    k = np.asarray(k, dtype=np.float32)
    v = np.asarray(v, dtype=np.float32)
    wq16 = np.asarray(Wq, dtype=np.float32).astype(np.float16)
    wk16 = np.asarray(Wk, dtype=np.float32).astype(np.float16)
    wv16 = np.asarray(Wv, dtype=np.float32).astype(np.float16)
    wo16 = np.asarray(Wo, dtype=np.float32).astype(np.float16)
    qTp = [_pack_stage(q[b].T.astype(np.float16)) for b in range(4)]
    kTp = [_pack_stage(k[b].T.astype(np.float16)) for b in range(4)]
    vTp = [_pack_stage(v[b].T.astype(np.float16)) for b in range(4)]
    in_maps = []
    for c in range(N_CORES):
        b, hh = c // 2, c % 2
        sl = slice(hh * HD, (hh + 1) * HD)
        in_maps.append(
            {
                "qTp": qTp[b],
                "kTp": kTp[b],
                "vTp": vTp[b],
                "wq": _pack_w_m(np.ascontiguousarray(wq16[:, sl])),
                "wk": _pack_w_m(np.ascontiguousarray(wk16[:, sl])),
                "wv": _pack_w(np.ascontiguousarray(wv16[:, sl])),
                "wo": _pack_wo(np.ascontiguousarray(wo16[sl, :])),
            }
        )
    return in_maps


def kernel(q, k, v, Wq, Wk, Wv, Wo):
    nc = _get_nc()
    in_maps = build_in_maps(q, k, v, Wq, Wk, Wv, Wo)
    res = run_bass_kernel_spmd(nc, in_maps, core_ids=list(range(N_CORES)))
    outs = [res.results[c]["out"].astype(np.float32) for c in range(N_CORES)]
    return np.stack([outs[2 * b] + outs[2 * b + 1] for b in range(4)], axis=0)


# revision 42
# speedup vs baseline: 1.0496x; 1.0293x over previous
"""Multi-head attention TRN2 Bass kernel (8 NeuronCores), v2.

Problem: B=4, S=2048, D_MODEL=1024, H=16, d_k=d_v=64 (fp32 in/out).

Sharding: core c handles batch b=c//2 and head-half hh=c%2 (8 heads).
Each core computes partial_out = softmax(qh@khT/8) @ vh @ Wo[rows of its
heads]; the host sums the two partials per batch.

v2 structure (vs v1): the kernel is ACT(exp)-throughput-paced in steady
state (exp of 33.5M scores/core = 284us on ScalarE vs ~274us of PE
work), so everything else must hide under the exp stream:
  - blocks run PAIR-major ((sq0..3, pair0), (sq0..3, pair1), ...): the
    first exp only needs K-proj m0 + Q-proj m0-sb0, so the exp stream
    starts ~8us in (v1: 71.6us).
  - all other PE work (V projection, K/Q projections for later pairs,
    Wo for completed sq blocks) is chopped into 1-2-matmul "filler
    units" pumped between the skt iterations, filling the PE slack the
    ACT pace leaves without delaying the next scores. Fillers
    accumulate in a dedicated 2x1-bank PSUM pool (ps_fill) so their
    slot lifetimes never collide with the block accumulators.
  - PSUM budget: scps 2x[128,1024] + av 1x[128,1024] + fill
    2x[128,512] = 16KB/partition exactly. av is single-buffered; the
    WAR wait on the previous block's normalize is absorbed by pumping
    fillers at the block boundary.
  - host pre-packs every input into the exact SBUF staging layout so
    each DMA is one fully contiguous 1MB transfer (8KB/partition
    lines), spread over the sync/scalar/vector DMA queues in
    consumption order.
  - the vh ones-region memset runs on the idle GpSimd engine instead of
    blocking the Vector engine's projection casts.
Per-block math (scores transposed, row-group-packed K=64 pairs; AV with
the [ones64|vh] stationary giving the softmax denominator for free;
reciprocal+mul normalize into fp16 Wo stationaries) is unchanged.
"""

import numpy as np

import concourse.bass as bass  # noqa: F401
import concourse.mybir as mybir
import concourse.tile as tile
from concourse import bacc
from concourse.bass_utils import run_bass_kernel_spmd

S = 2048  # sequence length
D = 1024  # d_model
HPC = 8  # heads per core
DK = 64  # head dim
HD = HPC * DK  # 512: projected width per core
N_CORES = 8

SB = S // 512  # 4 s-blocks of 512
KT = D // 128  # 8 contraction tiles for projections
SKT = S // 128  # 16 key tiles
NP = HPC // 2  # 4 head pairs
F32 = mybir.dt.float32
F16 = mybir.dt.float16

_CACHE = {}


def _build(debug=False):
    nc = bacc.Bacc("TRN2", target_bir_lowering=False, debug=False, num_devices=N_CORES)
    # host-packed staging layouts: [sb, 128, KT, 512] so each sb is one
    # contiguous 1MB DMA; weights pre-packed to their SBUF layouts.
    kTp = nc.dram_tensor("kTp", [SB, 128, KT, 512], F16, kind="ExternalInput")
    qTp = nc.dram_tensor("qTp", [SB, 128, KT, 512], F16, kind="ExternalInput")
    vTp = nc.dram_tensor("vTp", [SB, 128, KT, 512], F16, kind="ExternalInput")
    # wq/wk are m-major so the m0 slice is one contiguous 256KB DMA
    wq = nc.dram_tensor("wq", [NP, 128, KT, 128], F16, kind="ExternalInput")
    wk = nc.dram_tensor("wk", [NP, 128, KT, 128], F16, kind="ExternalInput")
    wv = nc.dram_tensor("wv", [128, KT, HD], F16, kind="ExternalInput")
    wo = nc.dram_tensor("wo", [128, HD // 128, D], F16, kind="ExternalInput")
    out = nc.dram_tensor("out", [S, D], F16, kind="ExternalOutput")
    if debug:
        dbg_stk = nc.dram_tensor("dbg_stk", [SB, NP, 128, 512], F16, kind="ExternalOutput")
        dbg_vh = nc.dram_tensor("dbg_vh", [128, SKT, HPC, 128], F16, kind="ExternalOutput")
        dbg_kh0 = nc.dram_tensor("dbg_kh0", [128, S], F16, kind="ExternalOutput")
        dbg_qh0 = nc.dram_tensor("dbg_qh0", [128, S], F16, kind="ExternalOutput")
        dbg_kh3 = nc.dram_tensor("dbg_kh3", [128, S], F16, kind="ExternalOutput")
        dbg_qh3 = nc.dram_tensor("dbg_qh3", [128, S], F16, kind="ExternalOutput")

    with tile.TileContext(nc) as tc:
        with (
            tc.tile_pool(name="resident", bufs=1) as resident,
            tc.tile_pool(name="tstage", bufs=12) as tstage,
            tc.tile_pool(name="khp", bufs=2) as khpool,
            tc.tile_pool(name="qhp", bufs=2) as qhpool,
            tc.tile_pool(name="et", bufs=3) as etp,
            tc.tile_pool(name="rcp", bufs=1) as rcpp,
            tc.tile_pool(name="stk", bufs=16) as stkp,
            tc.tile_pool(name="outst", bufs=2) as outstp,
            tc.tile_pool(name="ps_sc", bufs=2, space="PSUM") as ps_sc,
            tc.tile_pool(name="ps_av", bufs=1, space="PSUM") as ps_av,
            tc.tile_pool(name="ps_fill", bufs=2, space="PSUM") as ps_fill,
        ):
            # --- resident tiles ---
            wv16 = resident.tile([128, KT, HD], F16)
            wk16 = resident.tile([128, NP, KT, 128], F16)
            wq16 = resident.tile([128, NP, KT, 128], F16)
            wo16 = resident.tile([128, HD // 128, D], F16)
            # AV stationary: [..., 0:64] = 1.0 (denominator), [..., 64:128] = vh
            vh = resident.tile([128, SKT, HPC, 128], F16)

            # --- staging tiles + DMA issue (consumption order, 3 queues) ---
            # vsts FIRST: V staging dies after the pre-chain, so the
            # next four tstage allocations (pair-2/3 kh/qh tiles) recycle
            # these slots instead of the still-live k/q staging.
            vsts = [
                tstage.tile([128, KT, 512], F16, tag="tstage", name=f"vsts{i}")
                for i in range(SB)
            ]
            ksts = [
                tstage.tile([128, KT, 512], F16, tag="tstage", name=f"ksts{i}")
                for i in range(SB)
            ]
            qsts = [
                tstage.tile([128, KT, 512], F16, tag="tstage", name=f"qsts{i}")
                for i in range(SB)
            ]

            # Per-queue DMA bandwidth is ~110GB/s, so the ~11.5MB the
            # pre-chain + early blocks need is spread across all three
            # queues in deadline order (K/Q/V first-wave in parallel).
            nc.sync.dma_start(out=wk16[:, 0], in_=wk.ap()[0])
            nc.sync.dma_start(out=ksts[0], in_=kTp.ap()[0])
            nc.sync.dma_start(out=vsts[1], in_=vTp.ap()[1])
            nc.sync.dma_start(out=ksts[2], in_=kTp.ap()[2])
            for m in range(1, NP):
                nc.sync.dma_start(out=wk16[:, m], in_=wk.ap()[m])
            nc.scalar.dma_start(out=wv16, in_=wv.ap())
            nc.scalar.dma_start(out=vsts[0], in_=vTp.ap()[0])
            nc.scalar.dma_start(out=vsts[3], in_=vTp.ap()[3])
            nc.scalar.dma_start(out=ksts[1], in_=kTp.ap()[1])
            nc.scalar.dma_start(out=qsts[1], in_=qTp.ap()[1])
            nc.scalar.dma_start(out=wo16, in_=wo.ap())
            nc.gpsimd.dma_start(out=wq16[:, 0], in_=wq.ap()[0])
            nc.gpsimd.dma_start(out=vsts[2], in_=vTp.ap()[2])
            nc.gpsimd.dma_start(out=qsts[0], in_=qTp.ap()[0])
            nc.gpsimd.dma_start(out=ksts[3], in_=kTp.ap()[3])
            nc.gpsimd.dma_start(out=qsts[2], in_=qTp.ap()[2])
            nc.gpsimd.dma_start(out=qsts[3], in_=qTp.ap()[3])
            for m in range(1, NP):
                nc.gpsimd.dma_start(out=wq16[:, m], in_=wq.ap()[m])
            # PE warm-up: ~28 matmuls on a junk tile, discarded into a
            # ps_fill slot. They run while the first DMAs land (PE would
            # idle) and keep the HAM clock gate at full rate for the
            # real pre-chain (else its first ~3.4us run at half clock).
            junk = resident.tile([128, 512], F16)
            nc.vector.memset(junk[:, :], 0.5)
            wps = ps_fill.tile([128, 512], F32, tag="fill", name="warm_ps")
            for _ in range(24):
                nc.tensor.matmul(
                    wps[:, :],
                    lhsT=junk[:, 0:128],
                    rhs=junk[:, :],
                    start=True,
                    stop=True,
                )
            # ones memsets on the Vector engine (DVE-write -> PE-read deps
            # are the well-trodden path). DVE is idle until the first
            # projection CAST, and these finish by ~8us.
            for c in range(SKT):
                nc.vector.memset(vh[:, c, :, 0:DK], 1.0)

            # --- projection chunk emitters (filler generators) ---
            def kq_proj_chunk(sts, w16, dst, m, sb):
                """One (m, sb) K/Q projection: 8 MMs + CAST into dst
                [128, 2048] at cols sb*512. Yields every 2 MMs."""
                ps = ps_fill.tile([128, 512], F32, tag="fill")
                for t in range(KT):
                    nc.tensor.matmul(
                        ps[:, :],
                        lhsT=w16[:, m, t, :],
                        rhs=sts[sb][:, t, :],
                        start=(t == 0),
                        stop=(t == KT - 1),
                    )
                    if t % 2 == 1 and t < KT - 1:
                        yield
                nc.vector.tensor_copy(dst[:, sb * 512 : (sb + 1) * 512], ps[:, :])
                yield

            def v_chunk(c):
                """One vh chunk (sb=c//4, cc=c%4): 8 MMs + CAST into
                vh[:, c, :, 64:128]. Yields every 2 MMs."""
                sb, cc = c // 4, c % 4
                ps = ps_fill.tile([128, 512], F32, tag="fill")
                for t in range(KT):
                    nc.tensor.matmul(
                        ps[:, :],
                        lhsT=vsts[sb][:, t, cc * 128 : (cc + 1) * 128],
                        rhs=wv16[:, t, :],
                        start=(t == 0),
                        stop=(t == KT - 1),
                    )
                    if t % 2 == 1 and t < KT - 1:
                        yield
                nc.vector.tensor_copy(
                    vh[:, c, :, DK:128],
                    ps[:, :].rearrange("p (h d) -> p h d", h=HPC),
                )
                yield

            def wo_chunk(sq, chunk, stks):
                """One Wo half-chunk pipeline: 128 out rows, both nh
                halves; 8 MMs total, CAST+DMA per nh. Yields every MM."""
                outst = outstp.tile([128, 2, 512], F16)
                mrange = slice(chunk * 128, (chunk + 1) * 128)
                for nh in range(2):
                    wops = ps_fill.tile([128, 512], F32, tag="fill")
                    for pair in range(NP):
                        nc.tensor.matmul(
                            wops[:, :],
                            lhsT=stks[pair][:, mrange],
                            rhs=wo16[:, pair, nh * 512 : (nh + 1) * 512],
                            start=(pair == 0),
                            stop=(pair == NP - 1),
                        )
                        if pair % 2 == 1 and not (nh == 1 and pair == NP - 1):
                            yield
                    nc.vector.tensor_copy(outst[:, nh, :], wops[:, :])
                row0 = sq * 512 + chunk * 128
                nc.sync.dma_start(
                    out=out.ap()[row0 : row0 + 128, :],
                    in_=outst.rearrange("p a b -> p (a b)"),
                )
                yield

            # --- filler machinery ---
            fillers = []  # list of (gate_block_idx, generator)

            def pump(cur_block, budget=1):
                done, i = 0, 0
                while done < budget and i < len(fillers):
                    gate, gen = fillers[i]
                    if gate > cur_block:
                        i += 1
                        continue
                    try:
                        next(gen)
                        done += 1
                    except StopIteration:
                        fillers.pop(i)
                return done

            def flush():
                while fillers:
                    _, gen = fillers.pop(0)
                    for _ in gen:
                        pass

            # --- attention block (inline finish) ---
            def av_mms(et, skt, av, pair):
                for x in range(2):
                    nc.tensor.matmul(
                        av[:, x * 512 : (x + 1) * 512],
                        lhsT=vh[:, skt, 2 * pair + x, :],
                        rhs=et[:, x, :],
                        start=(skt == 0),
                        stop=(skt == SKT - 1),
                    )

            def attention_block(block_idx, sq, pair, kh, qh, stks_by_sq):
                cols = slice(sq * 512, (sq + 1) * 512)
                budget = 1
                av = ps_av.tile([128, 1024], F32, tag="av")
                prev = None
                for skt in range(SKT):
                    scps = ps_sc.tile([128, 1024], F32, tag="sc")
                    kcols = slice(skt * 128, (skt + 1) * 128)
                    nc.tensor.matmul(
                        scps[:, 0:512],
                        lhsT=kh[0:64, kcols],
                        rhs=qh[0:64, cols],
                        start=True,
                        stop=True,
                    )
                    nc.tensor.matmul(
                        scps[:, 512:1024],
                        lhsT=kh[64:128, kcols],
                        rhs=qh[64:128, cols],
                        start=True,
                        stop=True,
                    )
                    if prev is not None:
                        av_mms(*prev)
                    pump(block_idx, budget)
                    et = etp.tile([128, 2, 512], F16)
                    nc.scalar.activation(
                        et.rearrange("p a b -> p (a b)"),
                        scps[:, :],
                        mybir.ActivationFunctionType.Exp,
                        scale=1.0 / np.sqrt(DK).item(),
                    )
                    prev = (et, skt, av, pair)
                # fillers cover the wait for the block's last exp
                pump(block_idx, 2)
                av_mms(*prev)
                # normalize: 1/r (rows 0:64) -> scale out rows into fp16
                # Wo stationary
                rcp = rcpp.tile([128, 1024], F32, tag="rcp")
                nc.vector.reciprocal_approx_fast(out=rcp[0:64, :], in_=av[0:64, :])
                stk = stkp.tile([128, 512], F16, tag="stk")
                nc.vector.tensor_mul(stk[0:64, :], av[64:128, 0:512], rcp[0:64, 0:512])
                nc.vector.tensor_mul(
                    stk[64:128, :], av[64:128, 512:1024], rcp[0:64, 512:1024]
                )
                stks_by_sq[sq][pair] = stk
                # boundary pump: runs during the normalize chain, so the
                # single-buffered av slot is free again by the time the
                # next block's first AV matmul needs it. Bigger in the
                # pair-3 window so the Wo backlog drains before the tail.
                pump(block_idx, 6 if block_idx >= 12 else 3)

            # --- pre-chain: K m0 sb0 + Q m0 sb0, then the FULL V
            # projection. V must complete before pair0's AV matmuls
            # consume vh at the exp pace; projecting it up front costs
            # ~27us that overlaps the kT/qT DMA stream, and removes the
            # in-block starvation stalls (and the buffer-recycling
            # pressure they caused). ---
            khts = {}
            qhts = {}
            khts[0] = khpool.tile([128, S], F16, tag="khp", name="kht0")
            qhts[0] = qhpool.tile([128, S], F16, tag="qhp", name="qht0")
            # K first (kT0 is the first-wave DMA), V while its tiles
            # stream in, Q last (its deadline is right before the first
            # scores; its DMA rides the slow gpsimd queue)
            for _ in kq_proj_chunk(ksts, wk16, khts[0], 0, 0):
                pass
            for c in range(SKT):
                for _ in v_chunk(c):
                    pass
            for _ in kq_proj_chunk(qsts, wq16, qhts[0], 0, 0):
                pass

            # --- filler list (priority order == deadline order) ---
            # K m0 sb1-3 (block 0 consumes khT[0] cols at 1 sb per 4 skt
            # iters), Q m0 sb1-3 (blocks 1-3), then per pair m: K all sb
            # + Q sb0 (due at block 4m), then Q sb1-3 (blocks 4m+1..3).
            for sb in range(1, SB):
                fillers.append((0, kq_proj_chunk(ksts, wk16, khts[0], 0, sb)))
            for sb in range(1, SB):
                fillers.append((0, kq_proj_chunk(qsts, wq16, qhts[0], 0, sb)))

            def m_proj_stream(m):
                """K (all sb) + Q (sb0) projections for pair m. Pairs 0/1
                use the dedicated kh/qh pools; pairs 2/3 allocate from the
                tstage pool, recycling the (dead) vsts slots -- a fresh
                slot either way, so the projection CASTs never wait on an
                older pair's still-executing scores."""
                if m < 2:
                    kh = khpool.tile([128, S], F16, tag="khp", name=f"kht{m}")
                    qh = qhpool.tile([128, S], F16, tag="qhp", name=f"qht{m}")
                else:
                    kh = tstage.tile(
                        [128, KT, 512], F16, tag="tstage", name=f"kht{m}"
                    ).rearrange("p a b -> p (a b)")[:, 0:S]
                    qh = tstage.tile(
                        [128, KT, 512], F16, tag="tstage", name=f"qht{m}"
                    ).rearrange("p a b -> p (a b)")[:, 0:S]
                khts[m] = kh
                qhts[m] = qh
                for sb in range(SB):
                    yield from kq_proj_chunk(ksts, wk16, kh, m, sb)
                yield from kq_proj_chunk(qsts, wq16, qh, m, 0)

            def q_rest_stream(m):
                for sb in range(1, SB):
                    yield from kq_proj_chunk(qsts, wq16, qhts[m], m, sb)

            m_streams = {}
            for m in range(1, NP):
                g = m_proj_stream(m)
                m_streams[m] = g
                fillers.append((0, g))
                fillers.append((0, q_rest_stream(m)))

            def ensure_pair_ready(m):
                """Force-finish pair m's projection stream if the filler
                pump hasn't consumed it yet (safety net)."""
                if m == 0:
                    return
                g = m_streams[m]
                for i, (_, gen) in enumerate(fillers):
                    if gen is g:
                        for _ in g:
                            pass
                        fillers.pop(i)
                        break

            stks_by_sq = [[None] * NP for _ in range(SB)]

            # --- the block chain, pair-major ---
            block_idx = 0
            for pair in range(NP):
                ensure_pair_ready(pair)
                if debug and pair == 1:
                    # snapshot pair0's projections before slot recycling
                    nc.scalar.dma_start(out=dbg_kh0.ap(), in_=khts[0][:, :])
                    nc.scalar.dma_start(out=dbg_qh0.ap(), in_=qhts[0][:, :])
                for sq in range(SB):
                    if pair == NP - 1 and sq >= 1:
                        # previous sq's last pair is done: queue its Wo
                        # into the filler stream
                        for chunk in range(4):
                            fillers.append(
                                (block_idx, wo_chunk(sq - 1, chunk, stks_by_sq[sq - 1]))
                            )
                    attention_block(
                        block_idx, sq, pair, khts[pair], qhts[pair], stks_by_sq
                    )
                    block_idx += 1

            flush()
            # tail: Wo for the last sq block
            for chunk in range(4):
                for _ in wo_chunk(SB - 1, chunk, stks_by_sq[SB - 1]):
                    pass
            if debug:
                for sq in range(SB):
                    for pair in range(NP):
                        nc.sync.dma_start(
                            out=dbg_stk.ap()[sq, pair],
                            in_=stks_by_sq[sq][pair][:, :],
                        )
                nc.sync.dma_start(out=dbg_vh.ap(), in_=vh[:, :, :, :])
                nc.sync.dma_start(out=dbg_kh3.ap(), in_=khts[3][:, :])
                nc.sync.dma_start(out=dbg_qh3.ap(), in_=qhts[3][:, :])

    nc.compile()
    return nc


def _get_nc():
    if "nc" not in _CACHE:
        _CACHE["nc"] = _build()
    return _CACHE["nc"]


def _pack_stage(aT):
    """[D, S] fp16 -> [SB, 128, KT, 512] contiguous staging layout."""
    # row t*128+p, col sb*512+s  ->  [sb, p, t, s]
    a = aT.reshape(KT, 128, SB, 512)  # [t, p, sb, s]
    return np.ascontiguousarray(a.transpose(2, 1, 0, 3))


def _pack_w(w):
    """[D, HD] fp16 -> [128, KT, HD] (row t*128+p -> [p, t, :])."""
    return np.ascontiguousarray(w.reshape(KT, 128, HD).transpose(1, 0, 2))


def _pack_w_m(w):
    """[D, HD] fp16 -> [NP, 128, KT, 128] (m-major pair chunks)."""
    return np.ascontiguousarray(w.reshape(KT, 128, NP, 128).transpose(2, 1, 0, 3))


def _pack_wo(w):
    """[HD, D] fp16 -> [128, 4, D]."""
    return np.ascontiguousarray(w.reshape(HD // 128, 128, D).transpose(1, 0, 2))


def build_in_maps(q, k, v, Wq, Wk, Wv, Wo):
    """Host prep: shard, cast fp16, pre-transpose + pack to DMA layouts."""
    q = np.asarray(q, dtype=np.float32)
    k = np.asarray(k, dtype=np.float32)
    v = np.asarray(v, dtype=np.float32)
    wq16 = np.asarray(Wq, dtype=np.float32).astype(np.float16)
    wk16 = np.asarray(Wk, dtype=np.float32).astype(np.float16)
    wv16 = np.asarray(Wv, dtype=np.float32).astype(np.float16)
    wo16 = np.asarray(Wo, dtype=np.float32).astype(np.float16)
    qTp = [_pack_stage(q[b].T.astype(np.float16)) for b in range(4)]
    kTp = [_pack_stage(k[b].T.astype(np.float16)) for b in range(4)]
    vTp = [_pack_stage(v[b].T.astype(np.float16)) for b in range(4)]
    in_maps = []
    for c in range(N_CORES):
        b, hh = c // 2, c % 2
        sl = slice(hh * HD, (hh + 1) * HD)
        in_maps.append(
            {
                "qTp": qTp[b],
                "kTp": kTp[b],
                "vTp": vTp[b],
                "wq": _pack_w_m(np.ascontiguousarray(wq16[:, sl])),
                "wk": _pack_w_m(np.ascontiguousarray(wk16[:, sl])),
                "wv": _pack_w(np.ascontiguousarray(wv16[:, sl])),
                "wo": _pack_wo(np.ascontiguousarray(wo16[sl, :])),
            }
        )
    return in_maps


def kernel(q, k, v, Wq, Wk, Wv, Wo):
    nc = _get_nc()
    in_maps = build_in_maps(q, k, v, Wq, Wk, Wv, Wo)
    res = run_bass_kernel_spmd(nc, in_maps, core_ids=list(range(N_CORES)))
    outs = [res.results[c]["out"].astype(np.float32) for c in range(N_CORES)]
    return np.stack([outs[2 * b] + outs[2 * b + 1] for b in range(4)], axis=0)
